# revision 16
# baseline (speedup 1.0000x reference)
"""Trainium2 Bass kernel for AttributeAttentionModule.

y = attention over heads of QKV projections:
  Q = sa @ Wq.T + bq ; K = x @ Wk.T + bk ; V = x @ Wv.T + bv   (all [B, D])
  per-sample scores[h,g] = Q_h . K_g / 32 ; softmax over g ; out_h = sum_g w_hg V_g

Data-parallel over 8 NeuronCores (batch sharded). Algebraic restructure cuts
the tensor-engine work from 2.0 to 1.5 bf16-units per core:

  * K-mean drop (exact): softmax over g is shift-invariant, so scores only
    need K projected onto the 2D contrast space of the 3 heads. K's D x D
    pass becomes a D x 2dh pass against orthonormal Helmert contrasts
    Ck1 = (Wk_1-Wk_2)/sqrt2, Ck2 = (Wk_1+Wk_2-2Wk_3)/sqrt6;
    t_hg = Q_h.E_g is reconstructed (shifted so t_3 = 0, e3 = 1).
  * V mean/deviation split: out_h = Vbar + c1_h*H1 + c2_h*H2 with
    c1 = (w1-w2)/sqrt2, c2 = (w1+w2-2w3)/sqrt6. Vbar = x @ Mvbar.T runs in
    bf16 (it carries the full output magnitude); H1/H2 use the same Helmert
    contrasts of Wv in fp8 - their fp8 noise is damped by the small |c|
    (~0.2 rms). The orthonormal basis minimizes reconstruction noise.

Q and all contrast passes run in fp8 e4m3 DoubleRow (2 MACs/PE/cycle);
weights prescaled by 128 with the 1/128^2 folded into the softmax exp scale
and the 1/128 for V into the combine coefficients. Attention is fully fused
in SBUF: E is consumed into score dots as each PSUM chunk lands, H chunks
fold straight into the half-accumulators (c-major dev order) which DMA out
as soon as their second contrast lands. PSUM is double-buffered (4 tags x 2
bufs); weight tiles are deep-buffered; next group's first Q-weight tiles
prefetch during the mean pass; the last dev sweep runs batch-tile-major so
the final combine overlaps the last matmuls.
"""

import os
import sys

for _p in ("/opt/trn_rl_repo", "/root/.axon_site/_ro/trn_rl_repo"):
    if os.path.isdir(_p) and _p not in sys.path:
        sys.path.append(_p)

import numpy as np
import ml_dtypes
from contextlib import ExitStack

B = 16384
D = 3072
H = 3
DH = D // H          # 1024
NCORES = 8
P = 128              # partition tile
NO = 512             # matmul moving free dim (one PSUM bank of fp32)
NOT_Q = D // NO      # 6 output-column tiles for the Q pass
NOT_E = 2 * DH // NO  # 4 tiles for the K-contrast pass
NOT_M = DH // NO     # 2 tiles for the V-mean pass
NOT_D = 2 * DH // NO  # 4 tiles for the V-contrast pass
KT = D // P          # 24 contraction tiles of 128
K8 = KT // 2         # 12 fp8 DoubleRow contraction tiles of 256
WS = 128.0           # fp8 weight prescale (power of two, exact)
ESCALE = 1.0 / (32.0 * WS * WS)  # softmax exp scale: 1/sqrt(dh) / WS^2
R2 = float(1.0 / np.sqrt(2.0))
R6 = float(1.0 / np.sqrt(6.0))
GBT = 4              # batch tiles per weight-streaming group

E4 = ml_dtypes.float8_e4m3
BF = ml_dtypes.bfloat16

_CACHE = {}


def _build(bs=B // NCORES, gbt=GBT):
    import concourse.bass as bass
    import concourse.tile as tile
    from concourse import bacc, mybir

    f32 = mybir.dt.float32
    f8 = mybir.dt.float8e4
    bf16 = mybir.dt.bfloat16
    mult = mybir.AluOpType.mult
    add = mybir.AluOpType.add
    sub = mybir.AluOpType.subtract
    bypass = mybir.AluOpType.bypass
    Exp = mybir.ActivationFunctionType.Exp
    DR = mybir.MatmulPerfMode.DoubleRow

    nbt = bs // P        # batch tiles per core
    ng = nbt // gbt      # weight-stream groups

    nc = bacc.Bacc(
        "TRN2", target_bir_lowering=False, debug=False, num_devices=NCORES
    )

    # pre-tiled inputs (see kernel() for host layouts)
    sa8d = nc.dram_tensor("sa8", [nbt, P, K8, 2, P], f8, kind="ExternalInput").ap()
    x8d = nc.dram_tensor("x8", [nbt, P, K8, 2, P], f8, kind="ExternalInput").ap()
    x16d = nc.dram_tensor("x16", [nbt, P, KT, P], bf16, kind="ExternalInput").ap()
    wq8d = nc.dram_tensor("wq8", [NOT_Q, 4, P, 3, 2, NO], f8, kind="ExternalInput").ap()
    we8d = nc.dram_tensor("we8", [NOT_E, 4, P, 3, 2, NO], f8, kind="ExternalInput").ap()
    wm16d = nc.dram_tensor("wm16", [NOT_M, 8, P, 3, NO], bf16, kind="ExternalInput").ap()
    wd8d = nc.dram_tensor("wd8", [NOT_D, 4, P, 3, 2, NO], f8, kind="ExternalInput").ap()
    bqd = nc.dram_tensor("bq128", [P, D], bf16, kind="ExternalInput").ap()
    bed = nc.dram_tensor("be128", [P, 2 * DH], bf16, kind="ExternalInput").ap()
    bmd = nc.dram_tensor("bm", [P, DH], bf16, kind="ExternalInput").ap()
    bdd = nc.dram_tensor("bd128", [P, 2 * DH], bf16, kind="ExternalInput").ap()
    outd = nc.dram_tensor("out", [bs, D], f32, kind="ExternalOutput").ap()

    with tile.TileContext(nc) as tc, ExitStack() as ctx:
        sapool = ctx.enter_context(tc.tile_pool(name="sapool", bufs=1))
        x8pool = ctx.enter_context(tc.tile_pool(name="x8pool", bufs=1))
        x16pool = ctx.enter_context(tc.tile_pool(name="x16pool", bufs=1))
        qpool = ctx.enter_context(tc.tile_pool(name="qpool", bufs=1))
        mpool = ctx.enter_context(tc.tile_pool(name="mpool", bufs=1))
        accpool = ctx.enter_context(tc.tile_pool(name="accpool", bufs=1))
        wpool = ctx.enter_context(tc.tile_pool(name="wpool", bufs=9))
        wmpool = ctx.enter_context(tc.tile_pool(name="wmpool", bufs=8))
        prepool = ctx.enter_context(tc.tile_pool(name="prepool", bufs=4))
        bpool = ctx.enter_context(tc.tile_pool(name="bpool", bufs=1))
        pspool = ctx.enter_context(tc.tile_pool(name="psum", bufs=2, space="PSUM"))
        kocpool = ctx.enter_context(tc.tile_pool(name="kocp", bufs=4))
        prodpool = ctx.enter_context(tc.tile_pool(name="prodp", bufs=2))
        smallp = ctx.enter_context(tc.tile_pool(name="smallp", bufs=4))

        bias_loaded = False
        bias_t = {}
        pre_wq = None  # next group's o=0 Q-weight tiles, prefetched in mean pass

        for g in range(ng):
            bts = [g * gbt + i for i in range(gbt)]

            # activation loads for this group (sync queue: independent of
            # the weight stream on gpsimd so they land during prior passes).
            # Group 0's x8/x16 loads are deferred into the Q pass (issued on
            # the gpsimd stream) so the head isn't one giant DMA burst.
            saT, x8T, x16T = [], [], []
            for i, bt in enumerate(bts):
                t = sapool.tile([P, K8, 2, P], f8, tag=f"sa{i}", name=f"sa{i}")
                if g == 0:
                    # split across two queues so the first Q chains (which
                    # consume k8 in order) start after ~1/2 of the bytes
                    nc.sync.dma_start(t[:, 0 : K8 // 2], sa8d[bt, :, 0 : K8 // 2])
                    nc.scalar.dma_start(t[:, K8 // 2 :], sa8d[bt, :, K8 // 2 :])
                else:
                    nc.sync.dma_start(t[:], sa8d[bt])
                saT.append(t)
            if not bias_loaded:
                # only bq is needed early (first q_sink); the rest are
                # deferred into the Q pass to keep the head DMA-lean
                bias_loaded = True
                for nm, src, w in (
                    ("q", bqd, D),
                    ("e", bed, 2 * DH),
                    ("m", bmd, DH),
                    ("d", bdd, 2 * DH),
                ):
                    t = bpool.tile([P, w], bf16, tag=f"b{nm}", name=f"b{nm}")
                    if nm == "q":
                        nc.sync.dma_start(t[:], src[:])
                    bias_t[nm] = t
            for i, bt in enumerate(bts):
                t = x8pool.tile([P, K8, 2, P], f8, tag=f"x8{i}", name=f"x8{i}")
                if g > 0:
                    nc.sync.dma_start(t[:], x8d[bt])
                x8T.append(t)
            for i, bt in enumerate(bts):
                t = x16pool.tile([P, KT, P], bf16, tag=f"x16{i}", name=f"x16{i}")
                if g > 0:
                    nc.sync.dma_start(t[:], x16d[bt])
                x16T.append(t)

            qt = [
                qpool.tile([P, D], bf16, tag=f"q{i}", name=f"q{i}")
                for i in range(gbt)
            ]
            mt = [
                mpool.tile([P, DH], bf16, tag=f"m{i}", name=f"m{i}")
                for i in range(gbt)
            ]
            acc = [
                accpool.tile([P, H * NO], f32, tag=f"acc{i}", name=f"acc{i}")
                for i in range(gbt)
            ]
            u2 = [
                smallp.tile([P, 12], f32, tag=f"u2_{i}", name=f"u2_{i}")
                for i in range(gbt)
            ]
            c1 = [
                smallp.tile([P, H], f32, tag=f"c1_{i}", name=f"c1_{i}")
                for i in range(gbt)
            ]
            c2 = [
                smallp.tile([P, H], f32, tag=f"c2_{i}", name=f"c2_{i}")
                for i in range(gbt)
            ]

            def fp8_pass(wTd, nots, acts, sink, pre=None, post_sweep=None,
                         fast_start=False, bias=None):
                """One fp8 DoubleRow projection sweep; sink(i, o, ps) consumes
                each finished PSUM chunk. pre = prefetched o=0 weight tiles.
                fast_start: o=0 weight DMAs split per j across 3 queues and
                matmuls emitted i-progressive so the very first chain starts
                after ~0.5 MB of DMA instead of the whole sweep's inputs.
                bias: SBUF bias tile pre-copied into PSUM on the scalar
                engine (chains then run with start=False), so the sink can
                read PSUM directly and the vector queue never drains it."""
                for o in range(nots):
                    ps = [
                        pspool.tile([P, NO], f32, tag=f"ps{i}", name=f"ps{i}")
                        for i in range(gbt)
                    ]
                    st0 = bias is None
                    if not st0:
                        for i in range(gbt):
                            nc.scalar.copy(
                                ps[i][:], bias[:, o * NO : (o + 1) * NO]
                            )
                    if fast_start and o == 0:
                        wts = []
                        for kg in range(4):
                            wt = wpool.tile([P, 3, 2, NO], f8, tag="w8", name="w8")
                            for j, eng in ((0, nc.gpsimd), (1, nc.scalar), (2, nc.sync)):
                                eng.dma_start(wt[:, j], wTd[o, kg][:, j])
                            wts.append(wt)
                        for kg in range(4):
                            for i in range(gbt):
                                for j in range(3):
                                    k8 = kg * 3 + j
                                    nc.tensor.matmul(
                                        ps[i][:],
                                        acts[i][:, k8],
                                        wts[kg][:, j],
                                        start=(k8 == 0 and st0),
                                        stop=(k8 == K8 - 1),
                                        perf_mode=DR,
                                    )
                    else:
                        for kg in range(4):
                            if o == 0 and pre is not None:
                                wt = pre[kg]
                            else:
                                wt = wpool.tile([P, 3, 2, NO], f8, tag="w8", name="w8")
                                nc.gpsimd.dma_start(wt[:], wTd[o, kg])
                            for j in range(3):
                                k8 = kg * 3 + j
                                for i in range(gbt):
                                    nc.tensor.matmul(
                                        ps[i][:],
                                        acts[i][:, k8],
                                        wt[:, j],
                                        start=(k8 == 0 and st0),
                                        stop=(k8 == K8 - 1),
                                        perf_mode=DR,
                                    )
                    for i in range(gbt):
                        sink(i, o, ps[i])
                    if post_sweep is not None:
                        post_sweep(o)

            # ---- Q pass: oc = ps + 128*bq -> qt (bf16) ----
            def q_sink(i, o, ps):
                nc.vector.tensor_add(
                    qt[i][:, o * NO : (o + 1) * NO],
                    ps[:],
                    bias_t["q"][:, o * NO : (o + 1) * NO],
                )

            def q_post_sweep(o):
                # group 0 only: stagger the x activation and remaining bias
                # loads into the Q pass instead of bursting everything at t=0
                if o == 1:
                    for i, bt in enumerate(bts):
                        nc.gpsimd.dma_start(x8T[i][:], x8d[bt])
                elif o == 2:
                    nc.sync.dma_start(bias_t["e"][:], bed[:])
                elif o == 3:
                    for i, bt in enumerate(bts):
                        nc.gpsimd.dma_start(x16T[i][:], x16d[bt])
                elif o == 4:
                    nc.sync.dma_start(bias_t["m"][:], bmd[:])
                    nc.sync.dma_start(bias_t["d"][:], bdd[:])

            fp8_pass(
                wq8d, NOT_Q, saT, q_sink, pre=pre_wq,
                post_sweep=(q_post_sweep if g == 0 else None),
                fast_start=(g == 0),
            )
            pre_wq = None

            # ---- E pass: K-contrast chunks dotted with Q inline ----
            # o = (iv, c): contrast iv in {0,1}, column-half c in {0,1}.
            # u2 col layout: iv*6 + h*2 + c (c-pairs reduced after the pass).
            def e_sink(i, o, ps):
                iv, c = divmod(o, 2)
                # dots read the bias-preloaded PSUM chunk directly: no
                # drain op on the (nearly saturated) vector queue
                for h in range(H):
                    prod = prodpool.tile([P, NO], bf16, tag="prod", name="prod")
                    nc.vector.scalar_tensor_tensor(
                        prod[:],
                        qt[i][:, h * DH + c * NO : h * DH + (c + 1) * NO],
                        1.0,
                        ps[:],
                        op0=bypass,
                        op1=mult,
                        accum_out=u2[i][:, (iv * H + h) * 2 + c : (iv * H + h) * 2 + c + 1],
                    )

            fp8_pass(we8d, NOT_E, x8T, e_sink, bias=bias_t["e"])

            # ---- softmax on contrast scores (tiny) ----
            # v1 = t1 - t3 = ESC*(u1/sqrt2 + 3*u2/sqrt6)
            # v2 = t2 - t3 = ESC*(-u1/sqrt2 + 3*u2/sqrt6); e3 = exp(0) = 1
            # c1_h = (w1 - w2)/(sqrt2*WS); c2_h = (w1 + w2 - 2*w3)/(sqrt6*WS)
            for i in range(gbt):
                u = smallp.tile([P, 2 * H], f32, tag="u", name="u")
                nc.vector.tensor_reduce(
                    u[:],
                    u2[i][:].rearrange("p (ivh two) -> p ivh two", two=2),
                    axis=mybir.AxisListType.X,
                    op=add,
                )
                a = smallp.tile([P, H], f32, tag="a", name="a")
                nc.scalar.mul(a[:], u[:, 0:H], ESCALE * R2)
                v1 = smallp.tile([P, H], f32, tag="v1", name="v1")
                nc.vector.scalar_tensor_tensor(
                    v1[:], u[:, H : 2 * H], ESCALE * 3.0 * R6, a[:],
                    op0=mult, op1=add,
                )
                v2 = smallp.tile([P, H], f32, tag="v2", name="v2")
                nc.vector.scalar_tensor_tensor(
                    v2[:], u[:, H : 2 * H], ESCALE * 3.0 * R6, a[:],
                    op0=mult, op1=sub,
                )
                e1 = smallp.tile([P, H], f32, tag="e1", name="e1")
                nc.scalar.activation(e1[:], v1[:], Exp)
                e2 = smallp.tile([P, H], f32, tag="e2", name="e2")
                nc.scalar.activation(e2[:], v2[:], Exp)
                e12 = smallp.tile([P, H], f32, tag="e12", name="e12")
                nc.vector.tensor_add(e12[:], e1[:], e2[:])
                esum = smallp.tile([P, H], f32, tag="esum", name="esum")
                nc.vector.tensor_scalar_add(esum[:], e12[:], 1.0)
                rcp = smallp.tile([P, H], f32, tag="rcp", name="rcp")
                nc.vector.reciprocal(rcp[:], esum[:])
                d1 = smallp.tile([P, H], f32, tag="d1", name="d1")
                nc.vector.scalar_tensor_tensor(
                    d1[:], e1[:], 1.0, e2[:], op0=bypass, op1=sub
                )
                d2 = smallp.tile([P, H], f32, tag="d2", name="d2")
                nc.vector.tensor_scalar_sub(d2[:], e12[:], 2.0)
                nc.vector.scalar_tensor_tensor(
                    c1[i][:], d1[:], R2 / WS, rcp[:], op0=mult, op1=mult
                )
                nc.vector.scalar_tensor_tensor(
                    c2[i][:], d2[:], R6 / WS, rcp[:], op0=mult, op1=mult
                )

            # ---- V mean pass (bf16): mt = ps + bm; softmax overlaps it ----
            for o in range(NOT_M):
                ps = [
                    pspool.tile([P, NO], f32, tag=f"ps{i}", name=f"ps{i}")
                    for i in range(gbt)
                ]
                for kg in range(8):
                    wt = wmpool.tile([P, 3, NO], bf16, tag="wv", name="wv")
                    nc.gpsimd.dma_start(wt[:], wm16d[o, kg])
                    for j in range(3):
                        k = kg * 3 + j
                        for i in range(gbt):
                            nc.tensor.matmul(
                                ps[i][:],
                                x16T[i][:, k],
                                wt[:, j],
                                start=(k == 0),
                                stop=(k == KT - 1),
                            )
                # prefetch next group's first Q-weight tiles once the mean
                # weight stream is queued, so the V->Q boundary doesn't stall
                if o == 0 and g < ng - 1:
                    # dedicated pool: the dev pass keeps allocating from
                    # wpool's ring, which would overwrite these before use
                    pre_wq = []
                    for kg in range(4):
                        wt = prepool.tile([P, 3, 2, NO], f8, tag="w8pre", name="w8pre")
                        nc.gpsimd.dma_start(wt[:], wq8d[0, kg])
                        pre_wq.append(wt)
                for i in range(gbt):
                    nc.vector.tensor_add(
                        mt[i][:, o * NO : (o + 1) * NO],
                        ps[i][:],
                        bias_t["m"][:, o * NO : (o + 1) * NO],
                    )

            # ---- V dev pass (fp8): o = (c, iv) so each column-half's two
            # contrasts land back-to-back; acc holds one column-half for all
            # 3 heads and DMAs out as soon as its second contrast lands.
            def dev_combine(i, bt, o, ps):
                c, iv = divmod(o, 2)
                for h in range(H):
                    asl = acc[i][:, h * NO : (h + 1) * NO]
                    if iv == 0:
                        nc.vector.scalar_tensor_tensor(
                            asl,
                            ps[:],
                            c1[i][:, h : h + 1],
                            mt[i][:, c * NO : (c + 1) * NO],
                            op0=mult,
                            op1=add,
                        )
                    else:
                        nc.vector.scalar_tensor_tensor(
                            asl,
                            ps[:],
                            c2[i][:, h : h + 1],
                            asl,
                            op0=mult,
                            op1=add,
                        )
                        dma_eng = (nc.scalar, nc.sync, nc.scalar)[h]
                        dma_eng.dma_start(
                            outd[
                                bt * P : bt * P + P,
                                h * DH + c * NO : h * DH + (c + 1) * NO,
                            ],
                            asl,
                        )

            merged_tail = g == ng - 1
            for o in range(2 if merged_tail else NOT_D):
                ps = [
                    pspool.tile([P, NO], f32, tag=f"ps{i}", name=f"ps{i}")
                    for i in range(gbt)
                ]
                for i in range(gbt):
                    nc.scalar.copy(
                        ps[i][:], bias_t["d"][:, o * NO : (o + 1) * NO]
                    )
                for kg in range(4):
                    wt = wpool.tile([P, 3, 2, NO], f8, tag="w8", name="w8")
                    nc.gpsimd.dma_start(wt[:], wd8d[o, kg])
                    for j in range(3):
                        k8 = kg * 3 + j
                        for i in range(gbt):
                            nc.tensor.matmul(
                                ps[i][:],
                                x8T[i][:, k8],
                                wt[:, j],
                                start=False,
                                stop=(k8 == K8 - 1),
                                perf_mode=DR,
                            )
                for i, bt in enumerate(bts):
                    dev_combine(i, bt, o, ps[i])
            if merged_tail:
                # last group's c=1 half runs PAIR-of-batch-tiles-major across
                # both contrast sweeps: two interleaved PSUM chains keep the
                # PE at throughput (one chain alone is latency-bound), while
                # each pair's output chunks DMA out during the next pair's
                # matmuls. Only the last pair's combines trail the final
                # matmul.
                wts23 = []
                for o in (2, 3):
                    for kg in range(4):
                        wt = wpool.tile([P, 3, 2, NO], f8, tag="w8", name="w8")
                        nc.gpsimd.dma_start(wt[:], wd8d[o, kg])
                        wts23.append(wt)
                for i0 in range(0, gbt, 2):
                    pair = (i0, i0 + 1)
                    for oi, o in enumerate((2, 3)):
                        ps = {
                            i: pspool.tile([P, NO], f32, tag=f"ps{i}", name=f"ps{i}")
                            for i in pair
                        }
                        for i in pair:
                            nc.scalar.copy(
                                ps[i][:], bias_t["d"][:, o * NO : (o + 1) * NO]
                            )
                        for kg in range(4):
                            for j in range(3):
                                k8 = kg * 3 + j
                                for i in pair:
                                    nc.tensor.matmul(
                                        ps[i][:],
                                        x8T[i][:, k8],
                                        wts23[oi * 4 + kg][:, j],
                                        start=False,
                                        stop=(k8 == K8 - 1),
                                        perf_mode=DR,
                                    )
                        for i in pair:
                            dev_combine(i, bts[i], o, ps[i])

    nc.compile()
    return nc


def _get_nc(bs=B // NCORES, gbt=GBT):
    key = (bs, gbt)
    if key not in _CACHE:
        _CACHE[key] = _build(bs, gbt)
    return _CACHE[key]


def _tile_w8(wT, nots):
    """fp8 DR tiling: w8[o, kg, p, j, i, n] = wT[((kg*3+j)*2+i)*128+p, o*512+n]."""
    w6 = wT.reshape(4, 3, 2, P, nots, NO).transpose(4, 0, 3, 1, 2, 5)
    return np.ascontiguousarray(w6)


def _prep_weights(Wq, Wk, Wv, bq, bk, bv):
    """Pre-tile weights: Q full fp8; Helmert contrasts of Wk/Wv blocks in
    fp8 (scaled by WS); V-block mean in bf16. The V-contrast (and its bias)
    column tiles are permuted to (c, iv) order to match the dev pass."""
    ws = {}
    wqT = (np.asarray(Wq, dtype=np.float32).T * np.float32(WS)).astype(E4)
    ws["q"] = _tile_w8(wqT, NOT_Q)

    def contrasts(W):
        M = np.asarray(W, dtype=np.float32).reshape(H, DH, D)
        return np.concatenate(
            [(M[0] - M[1]) * np.float32(R2), (M[0] + M[1] - 2 * M[2]) * np.float32(R6)],
            axis=0,
        )  # [2*DH, D]

    weT = (contrasts(Wk).T * np.float32(WS)).astype(E4)  # [D, 2*DH]
    ws["e"] = _tile_w8(weT, NOT_E)

    Mv = np.asarray(Wv, dtype=np.float32).reshape(H, DH, D)
    wmT = np.ascontiguousarray(Mv.mean(axis=0).T).astype(BF)  # [D, DH]
    ws["m"] = np.ascontiguousarray(
        wmT.reshape(8, 3, P, NOT_M, NO).transpose(3, 0, 2, 1, 4)
    )

    wdT = (contrasts(Wv).T * np.float32(WS)).astype(E4)  # [D, 2*DH]
    wdT = np.concatenate(
        [wdT[:, 0:NO], wdT[:, DH : DH + NO], wdT[:, NO:DH], wdT[:, DH + NO :]], axis=1
    )  # col tiles reordered to (c, iv)
    ws["d"] = _tile_w8(wdT, NOT_D)

    def bcontrasts(b):
        b3 = np.asarray(b, dtype=np.float32).reshape(H, DH)
        return np.concatenate(
            [(b3[0] - b3[1]) * np.float32(R2), (b3[0] + b3[1] - 2 * b3[2]) * np.float32(R6)]
        )

    bb = {}
    bb["q"] = (np.asarray(bq, dtype=np.float32) * np.float32(WS)).astype(BF)
    bb["e"] = (bcontrasts(bk) * np.float32(WS)).astype(BF)
    bb["m"] = np.asarray(bv, dtype=np.float32).reshape(H, DH).mean(axis=0).astype(BF)
    bdv = bcontrasts(bv) * np.float32(WS)
    bb["d"] = np.concatenate(
        [bdv[0:NO], bdv[DH : DH + NO], bdv[NO:DH], bdv[DH + NO :]]
    ).astype(BF)
    for nm in bb:
        bb[nm] = np.ascontiguousarray(np.broadcast_to(bb[nm], (P, bb[nm].shape[0])))
    return ws, bb


def _prep_act8(a, bs):
    """fp8 DoubleRow: a8[bt, p, k8, i, b] = a[bt*128+b, (k8*2+i)*128+p]."""
    nbt = bs // P
    a8 = a.astype(E4).reshape(nbt, P, K8, 2, P).transpose(0, 4, 2, 3, 1)
    return np.ascontiguousarray(a8)


def _prep_act16(a, bs):
    """bf16: a16[bt, p, k, b] = a[bt*128+b, k*128+p]."""
    nbt = bs // P
    a16 = a.astype(BF).reshape(nbt, P, KT, P).transpose(0, 3, 2, 1)
    return np.ascontiguousarray(a16)


def _in_maps(x, sa, ws, bb, bs):
    maps = []
    for c in range(NCORES):
        r0 = c * bs
        maps.append(
            {
                "sa8": _prep_act8(sa[r0 : r0 + bs], bs),
                "x8": _prep_act8(x[r0 : r0 + bs], bs),
                "x16": _prep_act16(x[r0 : r0 + bs], bs),
                "wq8": ws["q"],
                "we8": ws["e"],
                "wm16": ws["m"],
                "wd8": ws["d"],
                "bq128": bb["q"],
                "be128": bb["e"],
                "bm": bb["m"],
                "bd128": bb["d"],
            }
        )
    return maps


def kernel(x, synthetic_attributes, Wq, bq, Wk, bk, Wv, bv, **_ignored):
    from concourse import bass_utils

    x = np.asarray(x, dtype=np.float32)
    sa = np.asarray(synthetic_attributes, dtype=np.float32)
    bs = x.shape[0] // NCORES

    ws, bb = _prep_weights(Wq, Wk, Wv, bq, bk, bv)
    nc = _get_nc(bs=bs)
    in_maps = _in_maps(x, sa, ws, bb, bs)

    res = bass_utils.run_bass_kernel_spmd(nc, in_maps, core_ids=list(range(NCORES)))
    out = np.concatenate([res.results[c]["out"] for c in range(NCORES)], axis=0)
    return out


# revision 18
# speedup vs baseline: 1.1820x; 1.1820x over previous
"""Trainium2 Bass kernel for AttributeAttentionModule.

y = attention over heads of QKV projections:
  Q = sa @ Wq.T + bq ; K = x @ Wk.T + bk ; V = x @ Wv.T + bv   (all [B, D])
  per-sample scores[h,g] = Q_h . K_g / 32 ; softmax over g ; out_h = sum_g w_hg V_g

Data-parallel over 8 NeuronCores (batch sharded). Algebraic restructure cuts
the tensor-engine work from 2.0 to 1.5 bf16-units per core:

  * K-mean drop (exact): softmax over g is shift-invariant, so scores only
    need K projected onto the 2D contrast space of the 3 heads. K's D x D
    pass becomes a D x 2dh pass against orthonormal Helmert contrasts
    Ck1 = (Wk_1-Wk_2)/sqrt2, Ck2 = (Wk_1+Wk_2-2Wk_3)/sqrt6;
    t_hg = Q_h.E_g is reconstructed (shifted so t_3 = 0, e3 = 1).
  * V mean/deviation split: out_h = Vbar + c1_h*H1 + c2_h*H2 with
    c1 = (w1-w2)/sqrt2, c2 = (w1+w2-2w3)/sqrt6. Vbar = x @ Mvbar.T runs in
    bf16 (it carries the full output magnitude); H1/H2 use the same Helmert
    contrasts of Wv in fp8 - their fp8 noise is damped by the small |c|
    (~0.2 rms). The orthonormal basis minimizes reconstruction noise.

Q and all contrast passes run in fp8 e4m3 DoubleRow (2 MACs/PE/cycle);
weights prescaled by 128 with the 1/128^2 folded into the softmax exp scale
and the 1/128 for V into the combine coefficients. Attention is fully fused
in SBUF: E is consumed into score dots as each PSUM chunk lands, H chunks
fold straight into the half-accumulators (c-major dev order) which DMA out
as soon as their second contrast lands. PSUM is double-buffered (4 tags x 2
bufs); weight tiles are deep-buffered; next group's first Q-weight tiles
prefetch during the mean pass; the last dev sweep runs batch-tile-major so
the final combine overlaps the last matmuls.
"""

import os
import sys

for _p in ("/opt/trn_rl_repo", "/root/.axon_site/_ro/trn_rl_repo"):
    if os.path.isdir(_p) and _p not in sys.path:
        sys.path.append(_p)

import numpy as np
import ml_dtypes
from contextlib import ExitStack

B = 16384
D = 3072
H = 3
DH = D // H          # 1024
NCORES = 8
P = 128              # partition tile
NO = 512             # matmul moving free dim (one PSUM bank of fp32)
NOT_Q = D // NO      # 6 output-column tiles for the Q pass
NOT_E = 2 * DH // NO  # 4 tiles for the K-contrast pass
NOT_M = DH // NO     # 2 tiles for the V-mean pass
NOT_D = 2 * DH // NO  # 4 tiles for the V-contrast pass
KT = D // P          # 24 contraction tiles of 128
K8 = KT // 2         # 12 fp8 DoubleRow contraction tiles of 256
WS = 128.0           # fp8 weight prescale (power of two, exact)
ESCALE = 1.0 / (32.0 * WS * WS)  # softmax exp scale: 1/sqrt(dh) / WS^2
R2 = float(1.0 / np.sqrt(2.0))
R6 = float(1.0 / np.sqrt(6.0))
GBT = 4              # batch tiles per weight-streaming group

E4 = ml_dtypes.float8_e4m3
BF = ml_dtypes.bfloat16

_CACHE = {}


def _build(bs=B // NCORES, gbt=GBT):
    import concourse.bass as bass
    import concourse.tile as tile
    from concourse import bacc, mybir

    f32 = mybir.dt.float32
    f8 = mybir.dt.float8e4
    bf16 = mybir.dt.bfloat16
    mult = mybir.AluOpType.mult
    add = mybir.AluOpType.add
    sub = mybir.AluOpType.subtract
    bypass = mybir.AluOpType.bypass
    Exp = mybir.ActivationFunctionType.Exp
    DR = mybir.MatmulPerfMode.DoubleRow

    nbt = bs // P        # batch tiles per core
    ng = nbt // gbt      # weight-stream groups

    nc = bacc.Bacc(
        "TRN2", target_bir_lowering=False, debug=False, num_devices=NCORES
    )

    # pre-tiled inputs (see kernel() for host layouts)
    sa8d = nc.dram_tensor("sa8", [nbt, P, K8, 2, P], f8, kind="ExternalInput").ap()
    x8d = nc.dram_tensor("x8", [nbt, P, K8, 2, P], f8, kind="ExternalInput").ap()
    x16d = nc.dram_tensor("x16", [nbt, P, KT, P], bf16, kind="ExternalInput").ap()
    wq8d = nc.dram_tensor("wq8", [NOT_Q, 4, P, 3, 2, NO], f8, kind="ExternalInput").ap()
    we8d = nc.dram_tensor("we8", [NOT_E, 4, P, 3, 2, NO], f8, kind="ExternalInput").ap()
    wm16d = nc.dram_tensor("wm16", [NOT_M, 8, P, 3, NO], bf16, kind="ExternalInput").ap()
    wd8d = nc.dram_tensor("wd8", [NOT_D, 4, P, 3, 2, NO], f8, kind="ExternalInput").ap()
    bqd = nc.dram_tensor("bq128", [P, D], bf16, kind="ExternalInput").ap()
    bed = nc.dram_tensor("be128", [P, 2 * DH], bf16, kind="ExternalInput").ap()
    bmd = nc.dram_tensor("bm", [P, DH], bf16, kind="ExternalInput").ap()
    bdd = nc.dram_tensor("bd128", [P, 2 * DH], bf16, kind="ExternalInput").ap()
    outd = nc.dram_tensor("out", [bs, D], f32, kind="ExternalOutput").ap()

    with tile.TileContext(nc) as tc, ExitStack() as ctx:
        sapool = ctx.enter_context(tc.tile_pool(name="sapool", bufs=1))
        x8pool = ctx.enter_context(tc.tile_pool(name="x8pool", bufs=1))
        x16pool = ctx.enter_context(tc.tile_pool(name="x16pool", bufs=1))
        qpool = ctx.enter_context(tc.tile_pool(name="qpool", bufs=1))
        mpool = ctx.enter_context(tc.tile_pool(name="mpool", bufs=1))
        accpool = ctx.enter_context(tc.tile_pool(name="accpool", bufs=1))
        wpool = ctx.enter_context(tc.tile_pool(name="wpool", bufs=9))
        wmpool = ctx.enter_context(tc.tile_pool(name="wmpool", bufs=8))
        prepool = ctx.enter_context(tc.tile_pool(name="prepool", bufs=4))
        bpool = ctx.enter_context(tc.tile_pool(name="bpool", bufs=1))
        pspool = ctx.enter_context(tc.tile_pool(name="psum", bufs=2, space="PSUM"))
        kocpool = ctx.enter_context(tc.tile_pool(name="kocp", bufs=4))
        prodpool = ctx.enter_context(tc.tile_pool(name="prodp", bufs=2))
        smallp = ctx.enter_context(tc.tile_pool(name="smallp", bufs=4))

        bias_loaded = False
        bias_t = {}
        pre_wq = None  # next group's o=0 Q-weight tiles, prefetched in mean pass

        for g in range(ng):
            bts = [g * gbt + i for i in range(gbt)]

            # activation loads for this group (sync queue: independent of
            # the weight stream on gpsimd so they land during prior passes).
            # Group 0's x8/x16 loads are deferred into the Q pass (issued on
            # the gpsimd stream) so the head isn't one giant DMA burst.
            saT, x8T, x16T = [], [], []
            for i, bt in enumerate(bts):
                t = sapool.tile([P, K8, 2, P], f8, tag=f"sa{i}", name=f"sa{i}")
                if g == 0:
                    # split across four queues so the first Q chains (which
                    # consume k8 in order) start after ~1/4 of the bytes
                    for q, eng in enumerate(
                        (nc.sync, nc.scalar, nc.sync, nc.scalar)
                    ):
                        k0, k1 = q * (K8 // 4), (q + 1) * (K8 // 4)
                        eng.dma_start(t[:, k0:k1], sa8d[bt, :, k0:k1])
                else:
                    nc.sync.dma_start(t[:], sa8d[bt])
                saT.append(t)
            if not bias_loaded:
                # only bq is needed early (first q_sink); the rest are
                # deferred into the Q pass to keep the head DMA-lean
                bias_loaded = True
                for nm, src, w in (
                    ("q", bqd, D),
                    ("e", bed, 2 * DH),
                    ("m", bmd, DH),
                    ("d", bdd, 2 * DH),
                ):
                    t = bpool.tile([P, w], bf16, tag=f"b{nm}", name=f"b{nm}")
                    if nm == "q":
                        nc.sync.dma_start(t[:], src[:])
                    bias_t[nm] = t
            for i, bt in enumerate(bts):
                t = x8pool.tile([P, K8, 2, P], f8, tag=f"x8{i}", name=f"x8{i}")
                if g > 0:
                    nc.sync.dma_start(t[:], x8d[bt])
                x8T.append(t)
            for i, bt in enumerate(bts):
                t = x16pool.tile([P, KT, P], bf16, tag=f"x16{i}", name=f"x16{i}")
                if g > 0:
                    nc.sync.dma_start(t[:], x16d[bt])
                x16T.append(t)

            qt = [
                qpool.tile([P, D], bf16, tag=f"q{i}", name=f"q{i}")
                for i in range(gbt)
            ]
            mt = [
                mpool.tile([P, DH], bf16, tag=f"m{i}", name=f"m{i}")
                for i in range(gbt)
            ]
            acc = [
                accpool.tile([P, H * NO], f32, tag=f"acc{i}", name=f"acc{i}")
                for i in range(gbt)
            ]
            u2 = [
                smallp.tile([P, 12], f32, tag=f"u2_{i}", name=f"u2_{i}")
                for i in range(gbt)
            ]
            c1 = [
                smallp.tile([P, H], f32, tag=f"c1_{i}", name=f"c1_{i}")
                for i in range(gbt)
            ]
            c2 = [
                smallp.tile([P, H], f32, tag=f"c2_{i}", name=f"c2_{i}")
                for i in range(gbt)
            ]

            def fp8_pass(wTd, nots, acts, sink, pre=None, post_sweep=None,
                         fast_start=False, bias=None):
                """One fp8 DoubleRow projection sweep; sink(i, o, ps) consumes
                each finished PSUM chunk. pre = prefetched o=0 weight tiles.
                fast_start: o=0 weight DMAs split per j across 3 queues and
                matmuls emitted i-progressive so the very first chain starts
                after ~0.5 MB of DMA instead of the whole sweep's inputs.
                bias: SBUF bias tile pre-copied into PSUM on the scalar
                engine (chains then run with start=False), so the sink can
                read PSUM directly and the vector queue never drains it."""
                for o in range(nots):
                    ps = [
                        pspool.tile([P, NO], f32, tag=f"ps{i}", name=f"ps{i}")
                        for i in range(gbt)
                    ]
                    st0 = bias is None
                    if not st0:
                        for i in range(gbt):
                            nc.scalar.copy(
                                ps[i][:], bias[:, o * NO : (o + 1) * NO]
                            )
                    if fast_start and o == 0:
                        wts = []
                        for kg in range(4):
                            wt = wpool.tile([P, 3, 2, NO], f8, tag="w8", name="w8")
                            for j, eng in ((0, nc.gpsimd), (1, nc.scalar), (2, nc.sync)):
                                eng.dma_start(wt[:, j], wTd[o, kg][:, j])
                            wts.append(wt)
                        for kg in range(4):
                            for i in range(gbt):
                                for j in range(3):
                                    k8 = kg * 3 + j
                                    nc.tensor.matmul(
                                        ps[i][:],
                                        acts[i][:, k8],
                                        wts[kg][:, j],
                                        start=(k8 == 0 and st0),
                                        stop=(k8 == K8 - 1),
                                        perf_mode=DR,
                                    )
                    else:
                        for kg in range(4):
                            if o == 0 and pre is not None:
                                wt = pre[kg]
                            else:
                                wt = wpool.tile([P, 3, 2, NO], f8, tag="w8", name="w8")
                                nc.gpsimd.dma_start(wt[:], wTd[o, kg])
                            for j in range(3):
                                k8 = kg * 3 + j
                                for i in range(gbt):
                                    nc.tensor.matmul(
                                        ps[i][:],
                                        acts[i][:, k8],
                                        wt[:, j],
                                        start=(k8 == 0 and st0),
                                        stop=(k8 == K8 - 1),
                                        perf_mode=DR,
                                    )
                    for i in range(gbt):
                        sink(i, o, ps[i])
                    if post_sweep is not None:
                        post_sweep(o)

            # ---- Q pass: oc = ps + 128*bq -> qt (bf16) ----
            def q_sink(i, o, ps):
                nc.vector.tensor_add(
                    qt[i][:, o * NO : (o + 1) * NO],
                    ps[:],
                    bias_t["q"][:, o * NO : (o + 1) * NO],
                )

            def q_post_sweep(o):
                # group 0 only: stagger the x activation and remaining bias
                # loads into the Q pass instead of bursting everything at t=0
                if o == 1:
                    for i, bt in enumerate(bts):
                        nc.gpsimd.dma_start(x8T[i][:], x8d[bt])
                elif o == 2:
                    nc.sync.dma_start(bias_t["e"][:], bed[:])
                elif o == 3:
                    for i, bt in enumerate(bts):
                        nc.gpsimd.dma_start(x16T[i][:], x16d[bt])
                elif o == 4:
                    nc.sync.dma_start(bias_t["m"][:], bmd[:])
                    nc.sync.dma_start(bias_t["d"][:], bdd[:])

            fp8_pass(
                wq8d, NOT_Q, saT, q_sink, pre=pre_wq,
                post_sweep=(q_post_sweep if g == 0 else None),
                fast_start=(g == 0),
            )
            pre_wq = None

            # ---- E pass: K-contrast chunks dotted with Q inline ----
            # o = (iv, c): contrast iv in {0,1}, column-half c in {0,1}.
            # u2 col layout: iv*6 + h*2 + c (c-pairs reduced after the pass).
            def e_sink(i, o, ps):
                iv, c = divmod(o, 2)
                # dots read the bias-preloaded PSUM chunk directly: no
                # drain op on the (nearly saturated) vector queue
                for h in range(H):
                    prod = prodpool.tile([P, NO], bf16, tag="prod", name="prod")
                    nc.vector.scalar_tensor_tensor(
                        prod[:],
                        qt[i][:, h * DH + c * NO : h * DH + (c + 1) * NO],
                        1.0,
                        ps[:],
                        op0=bypass,
                        op1=mult,
                        accum_out=u2[i][:, (iv * H + h) * 2 + c : (iv * H + h) * 2 + c + 1],
                    )

            fp8_pass(we8d, NOT_E, x8T, e_sink, bias=bias_t["e"])

            # ---- softmax on contrast scores (tiny) ----
            # v1 = t1 - t3 = ESC*(u1/sqrt2 + 3*u2/sqrt6)
            # v2 = t2 - t3 = ESC*(-u1/sqrt2 + 3*u2/sqrt6); e3 = exp(0) = 1
            # c1_h = (w1 - w2)/(sqrt2*WS); c2_h = (w1 + w2 - 2*w3)/(sqrt6*WS)
            for i in range(gbt):
                u = smallp.tile([P, 2 * H], f32, tag="u", name="u")
                nc.vector.tensor_reduce(
                    u[:],
                    u2[i][:].rearrange("p (ivh two) -> p ivh two", two=2),
                    axis=mybir.AxisListType.X,
                    op=add,
                )
                a = smallp.tile([P, H], f32, tag="a", name="a")
                nc.scalar.mul(a[:], u[:, 0:H], ESCALE * R2)
                v1 = smallp.tile([P, H], f32, tag="v1", name="v1")
                nc.vector.scalar_tensor_tensor(
                    v1[:], u[:, H : 2 * H], ESCALE * 3.0 * R6, a[:],
                    op0=mult, op1=add,
                )
                v2 = smallp.tile([P, H], f32, tag="v2", name="v2")
                nc.vector.scalar_tensor_tensor(
                    v2[:], u[:, H : 2 * H], ESCALE * 3.0 * R6, a[:],
                    op0=mult, op1=sub,
                )
                e1 = smallp.tile([P, H], f32, tag="e1", name="e1")
                nc.scalar.activation(e1[:], v1[:], Exp)
                e2 = smallp.tile([P, H], f32, tag="e2", name="e2")
                nc.scalar.activation(e2[:], v2[:], Exp)
                e12 = smallp.tile([P, H], f32, tag="e12", name="e12")
                nc.vector.tensor_add(e12[:], e1[:], e2[:])
                esum = smallp.tile([P, H], f32, tag="esum", name="esum")
                nc.vector.tensor_scalar_add(esum[:], e12[:], 1.0)
                rcp = smallp.tile([P, H], f32, tag="rcp", name="rcp")
                nc.vector.reciprocal(rcp[:], esum[:])
                d1 = smallp.tile([P, H], f32, tag="d1", name="d1")
                nc.vector.scalar_tensor_tensor(
                    d1[:], e1[:], 1.0, e2[:], op0=bypass, op1=sub
                )
                d2 = smallp.tile([P, H], f32, tag="d2", name="d2")
                nc.vector.tensor_scalar_sub(d2[:], e12[:], 2.0)
                nc.vector.scalar_tensor_tensor(
                    c1[i][:], d1[:], R2 / WS, rcp[:], op0=mult, op1=mult
                )
                nc.vector.scalar_tensor_tensor(
                    c2[i][:], d2[:], R6 / WS, rcp[:], op0=mult, op1=mult
                )

            # ---- V mean pass (bf16): mt = ps + bm; softmax overlaps it ----
            for o in range(NOT_M):
                ps = [
                    pspool.tile([P, NO], f32, tag=f"ps{i}", name=f"ps{i}")
                    for i in range(gbt)
                ]
                for kg in range(8):
                    wt = wmpool.tile([P, 3, NO], bf16, tag="wv", name="wv")
                    nc.gpsimd.dma_start(wt[:], wm16d[o, kg])
                    for j in range(3):
                        k = kg * 3 + j
                        for i in range(gbt):
                            nc.tensor.matmul(
                                ps[i][:],
                                x16T[i][:, k],
                                wt[:, j],
                                start=(k == 0),
                                stop=(k == KT - 1),
                            )
                # prefetch next group's first Q-weight tiles once the mean
                # weight stream is queued, so the V->Q boundary doesn't stall
                if o == 0 and g < ng - 1:
                    # dedicated pool: the dev pass keeps allocating from
                    # wpool's ring, which would overwrite these before use
                    pre_wq = []
                    for kg in range(4):
                        wt = prepool.tile([P, 3, 2, NO], f8, tag="w8pre", name="w8pre")
                        nc.gpsimd.dma_start(wt[:], wq8d[0, kg])
                        pre_wq.append(wt)
                for i in range(gbt):
                    nc.vector.tensor_add(
                        mt[i][:, o * NO : (o + 1) * NO],
                        ps[i][:],
                        bias_t["m"][:, o * NO : (o + 1) * NO],
                    )

            # ---- V dev pass (fp8): o = (c, iv) so each column-half's two
            # contrasts land back-to-back; acc holds one column-half for all
            # 3 heads and DMAs out as soon as its second contrast lands.
            def dev_combine(i, bt, o, ps):
                c, iv = divmod(o, 2)
                for h in range(H):
                    asl = acc[i][:, h * NO : (h + 1) * NO]
                    if iv == 0:
                        nc.vector.scalar_tensor_tensor(
                            asl,
                            ps[:],
                            c1[i][:, h : h + 1],
                            mt[i][:, c * NO : (c + 1) * NO],
                            op0=mult,
                            op1=add,
                        )
                    else:
                        nc.vector.scalar_tensor_tensor(
                            asl,
                            ps[:],
                            c2[i][:, h : h + 1],
                            asl,
                            op0=mult,
                            op1=add,
                        )
                        dma_eng = (nc.scalar, nc.sync, nc.scalar)[h]
                        dma_eng.dma_start(
                            outd[
                                bt * P : bt * P + P,
                                h * DH + c * NO : h * DH + (c + 1) * NO,
                            ],
                            asl,
                        )

            merged_tail = g == ng - 1
            for o in range(2 if merged_tail else NOT_D):
                ps = [
                    pspool.tile([P, NO], f32, tag=f"ps{i}", name=f"ps{i}")
                    for i in range(gbt)
                ]
                for i in range(gbt):
                    nc.scalar.copy(
                        ps[i][:], bias_t["d"][:, o * NO : (o + 1) * NO]
                    )
                for kg in range(4):
                    wt = wpool.tile([P, 3, 2, NO], f8, tag="w8", name="w8")
                    nc.gpsimd.dma_start(wt[:], wd8d[o, kg])
                    for j in range(3):
                        k8 = kg * 3 + j
                        for i in range(gbt):
                            nc.tensor.matmul(
                                ps[i][:],
                                x8T[i][:, k8],
                                wt[:, j],
                                start=False,
                                stop=(k8 == K8 - 1),
                                perf_mode=DR,
                            )
                for i, bt in enumerate(bts):
                    dev_combine(i, bt, o, ps[i])
            if merged_tail:
                # last group's c=1 half runs PAIR-of-batch-tiles-major across
                # both contrast sweeps: two interleaved PSUM chains keep the
                # PE at throughput (one chain alone is latency-bound), while
                # each pair's output chunks DMA out during the next pair's
                # matmuls. Only the last pair's combines trail the final
                # matmul.
                wts23 = []
                for o in (2, 3):
                    for kg in range(4):
                        wt = wpool.tile([P, 3, 2, NO], f8, tag="w8", name="w8")
                        nc.gpsimd.dma_start(wt[:], wd8d[o, kg])
                        wts23.append(wt)
                for i0 in range(0, gbt, 2):
                    pair = (i0, i0 + 1)
                    for oi, o in enumerate((2, 3)):
                        ps = {
                            i: pspool.tile([P, NO], f32, tag=f"ps{i}", name=f"ps{i}")
                            for i in pair
                        }
                        for i in pair:
                            nc.scalar.copy(
                                ps[i][:], bias_t["d"][:, o * NO : (o + 1) * NO]
                            )
                        for kg in range(4):
                            for j in range(3):
                                k8 = kg * 3 + j
                                for i in pair:
                                    nc.tensor.matmul(
                                        ps[i][:],
                                        x8T[i][:, k8],
                                        wts23[oi * 4 + kg][:, j],
                                        start=False,
                                        stop=(k8 == K8 - 1),
                                        perf_mode=DR,
                                    )
                        for i in pair:
                            dev_combine(i, bts[i], o, ps[i])

    nc.compile()
    return nc


def _get_nc(bs=B // NCORES, gbt=GBT):
    key = (bs, gbt)
    if key not in _CACHE:
        _CACHE[key] = _build(bs, gbt)
    return _CACHE[key]


def _tile_w8(wT, nots):
    """fp8 DR tiling: w8[o, kg, p, j, i, n] = wT[((kg*3+j)*2+i)*128+p, o*512+n]."""
    w6 = wT.reshape(4, 3, 2, P, nots, NO).transpose(4, 0, 3, 1, 2, 5)
    return np.ascontiguousarray(w6)


def _prep_weights(Wq, Wk, Wv, bq, bk, bv):
    """Pre-tile weights: Q full fp8; Helmert contrasts of Wk/Wv blocks in
    fp8 (scaled by WS); V-block mean in bf16. The V-contrast (and its bias)
    column tiles are permuted to (c, iv) order to match the dev pass."""
    ws = {}
    wqT = (np.asarray(Wq, dtype=np.float32).T * np.float32(WS)).astype(E4)
    ws["q"] = _tile_w8(wqT, NOT_Q)

    def contrasts(W):
        M = np.asarray(W, dtype=np.float32).reshape(H, DH, D)
        return np.concatenate(
            [(M[0] - M[1]) * np.float32(R2), (M[0] + M[1] - 2 * M[2]) * np.float32(R6)],
            axis=0,
        )  # [2*DH, D]

    weT = (contrasts(Wk).T * np.float32(WS)).astype(E4)  # [D, 2*DH]
    ws["e"] = _tile_w8(weT, NOT_E)

    Mv = np.asarray(Wv, dtype=np.float32).reshape(H, DH, D)
    wmT = np.ascontiguousarray(Mv.mean(axis=0).T).astype(BF)  # [D, DH]
    ws["m"] = np.ascontiguousarray(
        wmT.reshape(8, 3, P, NOT_M, NO).transpose(3, 0, 2, 1, 4)
    )

    wdT = (contrasts(Wv).T * np.float32(WS)).astype(E4)  # [D, 2*DH]
    wdT = np.concatenate(
        [wdT[:, 0:NO], wdT[:, DH : DH + NO], wdT[:, NO:DH], wdT[:, DH + NO :]], axis=1
    )  # col tiles reordered to (c, iv)
    ws["d"] = _tile_w8(wdT, NOT_D)

    def bcontrasts(b):
        b3 = np.asarray(b, dtype=np.float32).reshape(H, DH)
        return np.concatenate(
            [(b3[0] - b3[1]) * np.float32(R2), (b3[0] + b3[1] - 2 * b3[2]) * np.float32(R6)]
        )

    bb = {}
    bb["q"] = (np.asarray(bq, dtype=np.float32) * np.float32(WS)).astype(BF)
    bb["e"] = (bcontrasts(bk) * np.float32(WS)).astype(BF)
    bb["m"] = np.asarray(bv, dtype=np.float32).reshape(H, DH).mean(axis=0).astype(BF)
    bdv = bcontrasts(bv) * np.float32(WS)
    bb["d"] = np.concatenate(
        [bdv[0:NO], bdv[DH : DH + NO], bdv[NO:DH], bdv[DH + NO :]]
    ).astype(BF)
    for nm in bb:
        bb[nm] = np.ascontiguousarray(np.broadcast_to(bb[nm], (P, bb[nm].shape[0])))
    return ws, bb


def _prep_act8(a, bs):
    """fp8 DoubleRow: a8[bt, p, k8, i, b] = a[bt*128+b, (k8*2+i)*128+p]."""
    nbt = bs // P
    a8 = a.astype(E4).reshape(nbt, P, K8, 2, P).transpose(0, 4, 2, 3, 1)
    return np.ascontiguousarray(a8)


def _prep_act16(a, bs):
    """bf16: a16[bt, p, k, b] = a[bt*128+b, k*128+p]."""
    nbt = bs // P
    a16 = a.astype(BF).reshape(nbt, P, KT, P).transpose(0, 3, 2, 1)
    return np.ascontiguousarray(a16)


def _in_maps(x, sa, ws, bb, bs):
    maps = []
    for c in range(NCORES):
        r0 = c * bs
        maps.append(
            {
                "sa8": _prep_act8(sa[r0 : r0 + bs], bs),
                "x8": _prep_act8(x[r0 : r0 + bs], bs),
                "x16": _prep_act16(x[r0 : r0 + bs], bs),
                "wq8": ws["q"],
                "we8": ws["e"],
                "wm16": ws["m"],
                "wd8": ws["d"],
                "bq128": bb["q"],
                "be128": bb["e"],
                "bm": bb["m"],
                "bd128": bb["d"],
            }
        )
    return maps


def kernel(x, synthetic_attributes, Wq, bq, Wk, bk, Wv, bv, **_ignored):
    from concourse import bass_utils

    x = np.asarray(x, dtype=np.float32)
    sa = np.asarray(synthetic_attributes, dtype=np.float32)
    bs = x.shape[0] // NCORES

    ws, bb = _prep_weights(Wq, Wk, Wv, bq, bk, bv)
    nc = _get_nc(bs=bs)
    in_maps = _in_maps(x, sa, ws, bb, bs)

    res = bass_utils.run_bass_kernel_spmd(nc, in_maps, core_ids=list(range(NCORES)))
    out = np.concatenate([res.results[c]["out"] for c in range(NCORES)], axis=0)
    return out


# revision 19
# speedup vs baseline: 1.1931x; 1.0094x over previous
"""Trainium2 Bass kernel for AttributeAttentionModule.

y = attention over heads of QKV projections:
  Q = sa @ Wq.T + bq ; K = x @ Wk.T + bk ; V = x @ Wv.T + bv   (all [B, D])
  per-sample scores[h,g] = Q_h . K_g / 32 ; softmax over g ; out_h = sum_g w_hg V_g

Data-parallel over 8 NeuronCores (batch sharded). Algebraic restructure cuts
the tensor-engine work from 2.0 to 1.5 bf16-units per core:

  * K-mean drop (exact): softmax over g is shift-invariant, so scores only
    need K projected onto the 2D contrast space of the 3 heads. K's D x D
    pass becomes a D x 2dh pass against orthonormal Helmert contrasts
    Ck1 = (Wk_1-Wk_2)/sqrt2, Ck2 = (Wk_1+Wk_2-2Wk_3)/sqrt6;
    t_hg = Q_h.E_g is reconstructed (shifted so t_3 = 0, e3 = 1).
  * V mean/deviation split: out_h = Vbar + c1_h*H1 + c2_h*H2 with
    c1 = (w1-w2)/sqrt2, c2 = (w1+w2-2w3)/sqrt6. Vbar = x @ Mvbar.T runs in
    bf16 (it carries the full output magnitude); H1/H2 use the same Helmert
    contrasts of Wv in fp8 - their fp8 noise is damped by the small |c|
    (~0.2 rms). The orthonormal basis minimizes reconstruction noise.

Q and all contrast passes run in fp8 e4m3 DoubleRow (2 MACs/PE/cycle);
weights prescaled by 128 with the 1/128^2 folded into the softmax exp scale
and the 1/128 for V into the combine coefficients. Attention is fully fused
in SBUF: E is consumed into score dots as each PSUM chunk lands, H chunks
fold straight into the half-accumulators (c-major dev order) which DMA out
as soon as their second contrast lands. PSUM is double-buffered (4 tags x 2
bufs); weight tiles are deep-buffered; next group's first Q-weight tiles
prefetch during the mean pass; the last dev sweep runs batch-tile-major so
the final combine overlaps the last matmuls.
"""

import os
import sys

for _p in ("/opt/trn_rl_repo", "/root/.axon_site/_ro/trn_rl_repo"):
    if os.path.isdir(_p) and _p not in sys.path:
        sys.path.append(_p)

import numpy as np
import ml_dtypes
from contextlib import ExitStack

B = 16384
D = 3072
H = 3
DH = D // H          # 1024
NCORES = 8
P = 128              # partition tile
NO = 512             # matmul moving free dim (one PSUM bank of fp32)
NOT_Q = D // NO      # 6 output-column tiles for the Q pass
NOT_E = 2 * DH // NO  # 4 tiles for the K-contrast pass
NOT_M = DH // NO     # 2 tiles for the V-mean pass
NOT_D = 2 * DH // NO  # 4 tiles for the V-contrast pass
KT = D // P          # 24 contraction tiles of 128
K8 = KT // 2         # 12 fp8 DoubleRow contraction tiles of 256
WS = 128.0           # fp8 weight prescale (power of two, exact)
ESCALE = 1.0 / (32.0 * WS * WS)  # softmax exp scale: 1/sqrt(dh) / WS^2
R2 = float(1.0 / np.sqrt(2.0))
R6 = float(1.0 / np.sqrt(6.0))
GBT = 4              # batch tiles per weight-streaming group

E4 = ml_dtypes.float8_e4m3
BF = ml_dtypes.bfloat16

_CACHE = {}


def _build(bs=B // NCORES, gbt=GBT):
    import concourse.bass as bass
    import concourse.tile as tile
    from concourse import bacc, mybir

    f32 = mybir.dt.float32
    f8 = mybir.dt.float8e4
    bf16 = mybir.dt.bfloat16
    mult = mybir.AluOpType.mult
    add = mybir.AluOpType.add
    sub = mybir.AluOpType.subtract
    bypass = mybir.AluOpType.bypass
    Exp = mybir.ActivationFunctionType.Exp
    DR = mybir.MatmulPerfMode.DoubleRow

    nbt = bs // P        # batch tiles per core
    ng = nbt // gbt      # weight-stream groups

    nc = bacc.Bacc(
        "TRN2", target_bir_lowering=False, debug=False, num_devices=NCORES
    )

    # pre-tiled inputs (see kernel() for host layouts)
    sa8d = nc.dram_tensor("sa8", [nbt, P, K8, 2, P], f8, kind="ExternalInput").ap()
    x8d = nc.dram_tensor("x8", [nbt, P, K8, 2, P], f8, kind="ExternalInput").ap()
    x16d = nc.dram_tensor("x16", [nbt, P, KT, P], bf16, kind="ExternalInput").ap()
    wq8d = nc.dram_tensor("wq8", [NOT_Q, 4, P, 3, 2, NO], f8, kind="ExternalInput").ap()
    we8d = nc.dram_tensor("we8", [NOT_E, 4, P, 3, 2, NO], f8, kind="ExternalInput").ap()
    wm16d = nc.dram_tensor("wm16", [NOT_M, 8, P, 3, NO], bf16, kind="ExternalInput").ap()
    wd8d = nc.dram_tensor("wd8", [NOT_D, 4, P, 3, 2, NO], f8, kind="ExternalInput").ap()
    bqd = nc.dram_tensor("bq128", [P, D], bf16, kind="ExternalInput").ap()
    bed = nc.dram_tensor("be128", [P, 2 * DH], bf16, kind="ExternalInput").ap()
    bmd = nc.dram_tensor("bm", [P, DH], bf16, kind="ExternalInput").ap()
    bdd = nc.dram_tensor("bd128", [P, 2 * DH], bf16, kind="ExternalInput").ap()
    outd = nc.dram_tensor("out", [bs, D], f32, kind="ExternalOutput").ap()

    with tile.TileContext(nc) as tc, ExitStack() as ctx:
        sapool = ctx.enter_context(tc.tile_pool(name="sapool", bufs=1))
        x8pool = ctx.enter_context(tc.tile_pool(name="x8pool", bufs=1))
        x16pool = ctx.enter_context(tc.tile_pool(name="x16pool", bufs=1))
        qpool = ctx.enter_context(tc.tile_pool(name="qpool", bufs=1))
        mpool = ctx.enter_context(tc.tile_pool(name="mpool", bufs=1))
        accpool = ctx.enter_context(tc.tile_pool(name="accpool", bufs=1))
        wpool = ctx.enter_context(tc.tile_pool(name="wpool", bufs=9))
        wmpool = ctx.enter_context(tc.tile_pool(name="wmpool", bufs=8))
        prepool = ctx.enter_context(tc.tile_pool(name="prepool", bufs=4))
        bpool = ctx.enter_context(tc.tile_pool(name="bpool", bufs=1))
        pspool = ctx.enter_context(tc.tile_pool(name="psum", bufs=2, space="PSUM"))
        kocpool = ctx.enter_context(tc.tile_pool(name="kocp", bufs=4))
        prodpool = ctx.enter_context(tc.tile_pool(name="prodp", bufs=2))
        smallp = ctx.enter_context(tc.tile_pool(name="smallp", bufs=4))

        bias_loaded = False
        bias_t = {}
        pre_wq = None  # next group's o=0 Q-weight tiles, prefetched in mean pass

        for g in range(ng):
            bts = [g * gbt + i for i in range(gbt)]

            # activation loads for this group (sync queue: independent of
            # the weight stream on gpsimd so they land during prior passes).
            # Group 0's x8/x16 loads are deferred into the Q pass (issued on
            # the gpsimd stream) so the head isn't one giant DMA burst.
            saT, x8T, x16T = [], [], []
            for i, bt in enumerate(bts):
                t = sapool.tile([P, K8, 2, P], f8, tag=f"sa{i}", name=f"sa{i}")
                if g == 0:
                    # split across two queues so the first Q chains (which
                    # consume k8 in order) start after ~1/2 of the bytes
                    nc.sync.dma_start(t[:, 0 : K8 // 2], sa8d[bt, :, 0 : K8 // 2])
                    nc.scalar.dma_start(t[:, K8 // 2 :], sa8d[bt, :, K8 // 2 :])
                else:
                    nc.sync.dma_start(t[:], sa8d[bt])
                saT.append(t)
            if not bias_loaded:
                # only bq is needed early (first q_sink); the rest are
                # deferred into the Q pass to keep the head DMA-lean
                bias_loaded = True
                for nm, src, w in (
                    ("q", bqd, D),
                    ("e", bed, 2 * DH),
                    ("m", bmd, DH),
                    ("d", bdd, 2 * DH),
                ):
                    t = bpool.tile([P, w], bf16, tag=f"b{nm}", name=f"b{nm}")
                    if nm == "q":
                        nc.sync.dma_start(t[:], src[:])
                    bias_t[nm] = t
            for i, bt in enumerate(bts):
                t = x8pool.tile([P, K8, 2, P], f8, tag=f"x8{i}", name=f"x8{i}")
                if g > 0:
                    nc.sync.dma_start(t[:], x8d[bt])
                x8T.append(t)
            for i, bt in enumerate(bts):
                t = x16pool.tile([P, KT, P], bf16, tag=f"x16{i}", name=f"x16{i}")
                if g > 0:
                    nc.sync.dma_start(t[:], x16d[bt])
                x16T.append(t)

            qt = [
                qpool.tile([P, D], bf16, tag=f"q{i}", name=f"q{i}")
                for i in range(gbt)
            ]
            mt = [
                mpool.tile([P, DH], bf16, tag=f"m{i}", name=f"m{i}")
                for i in range(gbt)
            ]
            acc = [
                accpool.tile([P, H * NO], f32, tag=f"acc{i}", name=f"acc{i}")
                for i in range(gbt)
            ]
            u2 = [
                smallp.tile([P, 12], f32, tag=f"u2_{i}", name=f"u2_{i}")
                for i in range(gbt)
            ]
            c1 = [
                smallp.tile([P, H], f32, tag=f"c1_{i}", name=f"c1_{i}")
                for i in range(gbt)
            ]
            c2 = [
                smallp.tile([P, H], f32, tag=f"c2_{i}", name=f"c2_{i}")
                for i in range(gbt)
            ]

            def fp8_pass(wTd, nots, acts, sink, pre=None, post_sweep=None,
                         fast_start=False, bias=None):
                """One fp8 DoubleRow projection sweep; sink(i, o, ps) consumes
                each finished PSUM chunk. pre = prefetched o=0 weight tiles.
                fast_start: o=0 weight DMAs split per j across 3 queues and
                matmuls emitted i-progressive so the very first chain starts
                after ~0.5 MB of DMA instead of the whole sweep's inputs.
                bias: SBUF bias tile pre-copied into PSUM on the scalar
                engine (chains then run with start=False), so the sink can
                read PSUM directly and the vector queue never drains it."""
                for o in range(nots):
                    ps = [
                        pspool.tile([P, NO], f32, tag=f"ps{i}", name=f"ps{i}")
                        for i in range(gbt)
                    ]
                    st0 = bias is None
                    if not st0:
                        for i in range(gbt):
                            nc.scalar.copy(
                                ps[i][:], bias[:, o * NO : (o + 1) * NO]
                            )
                    if fast_start and o == 0:
                        wts = []
                        for kg in range(4):
                            wt = wpool.tile([P, 3, 2, NO], f8, tag="w8", name="w8")
                            for j, eng in ((0, nc.gpsimd), (1, nc.scalar), (2, nc.sync)):
                                eng.dma_start(wt[:, j], wTd[o, kg][:, j])
                            wts.append(wt)
                        for kg in range(4):
                            for i in range(gbt):
                                for j in range(3):
                                    k8 = kg * 3 + j
                                    nc.tensor.matmul(
                                        ps[i][:],
                                        acts[i][:, k8],
                                        wts[kg][:, j],
                                        start=(k8 == 0 and st0),
                                        stop=(k8 == K8 - 1),
                                        perf_mode=DR,
                                    )
                    else:
                        for kg in range(4):
                            if o == 0 and pre is not None:
                                wt = pre[kg]
                            else:
                                wt = wpool.tile([P, 3, 2, NO], f8, tag="w8", name="w8")
                                nc.gpsimd.dma_start(wt[:], wTd[o, kg])
                            for j in range(3):
                                k8 = kg * 3 + j
                                for i in range(gbt):
                                    nc.tensor.matmul(
                                        ps[i][:],
                                        acts[i][:, k8],
                                        wt[:, j],
                                        start=(k8 == 0 and st0),
                                        stop=(k8 == K8 - 1),
                                        perf_mode=DR,
                                    )
                    for i in range(gbt):
                        sink(i, o, ps[i])
                    if post_sweep is not None:
                        post_sweep(o)

            # ---- Q pass: oc = ps + 128*bq -> qt (bf16) ----
            def q_sink(i, o, ps):
                nc.vector.tensor_add(
                    qt[i][:, o * NO : (o + 1) * NO],
                    ps[:],
                    bias_t["q"][:, o * NO : (o + 1) * NO],
                )

            def q_post_sweep(o):
                # group 0 only: stagger the x activation and remaining bias
                # loads into the Q pass instead of bursting everything at t=0
                if o == 1:
                    for i, bt in enumerate(bts):
                        nc.gpsimd.dma_start(x8T[i][:], x8d[bt])
                elif o == 2:
                    nc.sync.dma_start(bias_t["e"][:], bed[:])
                elif o == 3:
                    for i, bt in enumerate(bts):
                        nc.gpsimd.dma_start(x16T[i][:], x16d[bt])
                elif o == 4:
                    nc.sync.dma_start(bias_t["m"][:], bmd[:])
                    nc.sync.dma_start(bias_t["d"][:], bdd[:])

            fp8_pass(
                wq8d, NOT_Q, saT, q_sink, pre=pre_wq,
                post_sweep=(q_post_sweep if g == 0 else None),
                fast_start=(g == 0),
            )
            pre_wq = None

            # ---- E pass: K-contrast chunks dotted with Q inline ----
            # o = (iv, c): contrast iv in {0,1}, column-half c in {0,1}.
            # u2 col layout: iv*6 + h*2 + c (c-pairs reduced after the pass).
            def e_sink(i, o, ps):
                iv, c = divmod(o, 2)
                # dots read the bias-preloaded PSUM chunk directly: no
                # drain op on the (nearly saturated) vector queue
                for h in range(H):
                    prod = prodpool.tile([P, NO], bf16, tag="prod", name="prod")
                    nc.vector.scalar_tensor_tensor(
                        prod[:],
                        qt[i][:, h * DH + c * NO : h * DH + (c + 1) * NO],
                        1.0,
                        ps[:],
                        op0=bypass,
                        op1=mult,
                        accum_out=u2[i][:, (iv * H + h) * 2 + c : (iv * H + h) * 2 + c + 1],
                    )

            fp8_pass(we8d, NOT_E, x8T, e_sink, bias=bias_t["e"])

            # ---- softmax on contrast scores (tiny) ----
            # v1 = t1 - t3 = ESC*(u1/sqrt2 + 3*u2/sqrt6)
            # v2 = t2 - t3 = ESC*(-u1/sqrt2 + 3*u2/sqrt6); e3 = exp(0) = 1
            # c1_h = (w1 - w2)/(sqrt2*WS); c2_h = (w1 + w2 - 2*w3)/(sqrt6*WS)
            for i in range(gbt):
                u = smallp.tile([P, 2 * H], f32, tag="u", name="u")
                nc.vector.tensor_reduce(
                    u[:],
                    u2[i][:].rearrange("p (ivh two) -> p ivh two", two=2),
                    axis=mybir.AxisListType.X,
                    op=add,
                )
                a = smallp.tile([P, H], f32, tag="a", name="a")
                nc.scalar.mul(a[:], u[:, 0:H], ESCALE * R2)
                v1 = smallp.tile([P, H], f32, tag="v1", name="v1")
                nc.vector.scalar_tensor_tensor(
                    v1[:], u[:, H : 2 * H], ESCALE * 3.0 * R6, a[:],
                    op0=mult, op1=add,
                )
                v2 = smallp.tile([P, H], f32, tag="v2", name="v2")
                nc.vector.scalar_tensor_tensor(
                    v2[:], u[:, H : 2 * H], ESCALE * 3.0 * R6, a[:],
                    op0=mult, op1=sub,
                )
                e1 = smallp.tile([P, H], f32, tag="e1", name="e1")
                nc.scalar.activation(e1[:], v1[:], Exp)
                e2 = smallp.tile([P, H], f32, tag="e2", name="e2")
                nc.scalar.activation(e2[:], v2[:], Exp)
                e12 = smallp.tile([P, H], f32, tag="e12", name="e12")
                nc.vector.tensor_add(e12[:], e1[:], e2[:])
                esum = smallp.tile([P, H], f32, tag="esum", name="esum")
                nc.vector.tensor_scalar_add(esum[:], e12[:], 1.0)
                rcp = smallp.tile([P, H], f32, tag="rcp", name="rcp")
                nc.vector.reciprocal(rcp[:], esum[:])
                d1 = smallp.tile([P, H], f32, tag="d1", name="d1")
                nc.vector.scalar_tensor_tensor(
                    d1[:], e1[:], 1.0, e2[:], op0=bypass, op1=sub
                )
                d2 = smallp.tile([P, H], f32, tag="d2", name="d2")
                nc.vector.tensor_scalar_sub(d2[:], e12[:], 2.0)
                nc.vector.scalar_tensor_tensor(
                    c1[i][:], d1[:], R2 / WS, rcp[:], op0=mult, op1=mult
                )
                nc.vector.scalar_tensor_tensor(
                    c2[i][:], d2[:], R6 / WS, rcp[:], op0=mult, op1=mult
                )

            # ---- V mean pass (bf16): mt = ps + bm; softmax overlaps it ----
            for o in range(NOT_M):
                ps = [
                    pspool.tile([P, NO], f32, tag=f"ps{i}", name=f"ps{i}")
                    for i in range(gbt)
                ]
                for kg in range(8):
                    wt = wmpool.tile([P, 3, NO], bf16, tag="wv", name="wv")
                    nc.gpsimd.dma_start(wt[:], wm16d[o, kg])
                    for j in range(3):
                        k = kg * 3 + j
                        for i in range(gbt):
                            nc.tensor.matmul(
                                ps[i][:],
                                x16T[i][:, k],
                                wt[:, j],
                                start=(k == 0),
                                stop=(k == KT - 1),
                            )
                # prefetch next group's first Q-weight tiles once the mean
                # weight stream is queued, so the V->Q boundary doesn't stall
                if o == 0 and g < ng - 1:
                    # dedicated pool: the dev pass keeps allocating from
                    # wpool's ring, which would overwrite these before use
                    pre_wq = []
                    for kg in range(4):
                        wt = prepool.tile([P, 3, 2, NO], f8, tag="w8pre", name="w8pre")
                        nc.gpsimd.dma_start(wt[:], wq8d[0, kg])
                        pre_wq.append(wt)
                for i in range(gbt):
                    nc.vector.tensor_add(
                        mt[i][:, o * NO : (o + 1) * NO],
                        ps[i][:],
                        bias_t["m"][:, o * NO : (o + 1) * NO],
                    )

            # ---- V dev pass (fp8): o = (c, iv) so each column-half's two
            # contrasts land back-to-back; acc holds one column-half for all
            # 3 heads and DMAs out as soon as its second contrast lands.
            def dev_combine(i, bt, o, ps):
                c, iv = divmod(o, 2)
                for h in range(H):
                    asl = acc[i][:, h * NO : (h + 1) * NO]
                    if iv == 0:
                        nc.vector.scalar_tensor_tensor(
                            asl,
                            ps[:],
                            c1[i][:, h : h + 1],
                            mt[i][:, c * NO : (c + 1) * NO],
                            op0=mult,
                            op1=add,
                        )
                    else:
                        nc.vector.scalar_tensor_tensor(
                            asl,
                            ps[:],
                            c2[i][:, h : h + 1],
                            asl,
                            op0=mult,
                            op1=add,
                        )
                        dma_eng = (nc.scalar, nc.sync, nc.scalar)[h]
                        dma_eng.dma_start(
                            outd[
                                bt * P : bt * P + P,
                                h * DH + c * NO : h * DH + (c + 1) * NO,
                            ],
                            asl,
                        )

            merged_tail = g == ng - 1
            for o in range(2 if merged_tail else NOT_D):
                ps = [
                    pspool.tile([P, NO], f32, tag=f"ps{i}", name=f"ps{i}")
                    for i in range(gbt)
                ]
                for i in range(gbt):
                    nc.scalar.copy(
                        ps[i][:], bias_t["d"][:, o * NO : (o + 1) * NO]
                    )
                for kg in range(4):
                    wt = wpool.tile([P, 3, 2, NO], f8, tag="w8", name="w8")
                    nc.gpsimd.dma_start(wt[:], wd8d[o, kg])
                    for j in range(3):
                        k8 = kg * 3 + j
                        for i in range(gbt):
                            nc.tensor.matmul(
                                ps[i][:],
                                x8T[i][:, k8],
                                wt[:, j],
                                start=False,
                                stop=(k8 == K8 - 1),
                                perf_mode=DR,
                            )
                for i, bt in enumerate(bts):
                    dev_combine(i, bt, o, ps[i])
            if merged_tail:
                # last group's c=1 half runs PAIR-of-batch-tiles-major across
                # both contrast sweeps: two interleaved PSUM chains keep the
                # PE at throughput (one chain alone is latency-bound), while
                # each pair's output chunks DMA out during the next pair's
                # matmuls. Only the last pair's combines trail the final
                # matmul.
                wts23 = []
                for o in (2, 3):
                    for kg in range(4):
                        wt = wpool.tile([P, 3, 2, NO], f8, tag="w8", name="w8")
                        nc.gpsimd.dma_start(wt[:], wd8d[o, kg])
                        wts23.append(wt)
                for i0 in range(0, gbt, 2):
                    pair = (i0, i0 + 1)
                    for oi, o in enumerate((2, 3)):
                        ps = {
                            i: pspool.tile([P, NO], f32, tag=f"ps{i}", name=f"ps{i}")
                            for i in pair
                        }
                        for i in pair:
                            nc.scalar.copy(
                                ps[i][:], bias_t["d"][:, o * NO : (o + 1) * NO]
                            )
                        for kg in range(4):
                            for j in range(3):
                                k8 = kg * 3 + j
                                for i in pair:
                                    nc.tensor.matmul(
                                        ps[i][:],
                                        x8T[i][:, k8],
                                        wts23[oi * 4 + kg][:, j],
                                        start=False,
                                        stop=(k8 == K8 - 1),
                                        perf_mode=DR,
                                    )
                        for i in pair:
                            dev_combine(i, bts[i], o, ps[i])

    nc.compile()
    return nc


def _get_nc(bs=B // NCORES, gbt=GBT):
    key = (bs, gbt)
    if key not in _CACHE:
        _CACHE[key] = _build(bs, gbt)
    return _CACHE[key]


def _tile_w8(wT, nots):
    """fp8 DR tiling: w8[o, kg, p, j, i, n] = wT[((kg*3+j)*2+i)*128+p, o*512+n]."""
    w6 = wT.reshape(4, 3, 2, P, nots, NO).transpose(4, 0, 3, 1, 2, 5)
    return np.ascontiguousarray(w6)


def _prep_weights(Wq, Wk, Wv, bq, bk, bv):
    """Pre-tile weights: Q full fp8; Helmert contrasts of Wk/Wv blocks in
    fp8 (scaled by WS); V-block mean in bf16. The V-contrast (and its bias)
    column tiles are permuted to (c, iv) order to match the dev pass."""
    ws = {}
    wqT = (np.asarray(Wq, dtype=np.float32).T * np.float32(WS)).astype(E4)
    ws["q"] = _tile_w8(wqT, NOT_Q)

    def contrasts(W):
        M = np.asarray(W, dtype=np.float32).reshape(H, DH, D)
        return np.concatenate(
            [(M[0] - M[1]) * np.float32(R2), (M[0] + M[1] - 2 * M[2]) * np.float32(R6)],
            axis=0,
        )  # [2*DH, D]

    weT = (contrasts(Wk).T * np.float32(WS)).astype(E4)  # [D, 2*DH]
    ws["e"] = _tile_w8(weT, NOT_E)

    Mv = np.asarray(Wv, dtype=np.float32).reshape(H, DH, D)
    wmT = np.ascontiguousarray(Mv.mean(axis=0).T).astype(BF)  # [D, DH]
    ws["m"] = np.ascontiguousarray(
        wmT.reshape(8, 3, P, NOT_M, NO).transpose(3, 0, 2, 1, 4)
    )

    wdT = (contrasts(Wv).T * np.float32(WS)).astype(E4)  # [D, 2*DH]
    wdT = np.concatenate(
        [wdT[:, 0:NO], wdT[:, DH : DH + NO], wdT[:, NO:DH], wdT[:, DH + NO :]], axis=1
    )  # col tiles reordered to (c, iv)
    ws["d"] = _tile_w8(wdT, NOT_D)

    def bcontrasts(b):
        b3 = np.asarray(b, dtype=np.float32).reshape(H, DH)
        return np.concatenate(
            [(b3[0] - b3[1]) * np.float32(R2), (b3[0] + b3[1] - 2 * b3[2]) * np.float32(R6)]
        )

    bb = {}
    bb["q"] = (np.asarray(bq, dtype=np.float32) * np.float32(WS)).astype(BF)
    bb["e"] = (bcontrasts(bk) * np.float32(WS)).astype(BF)
    bb["m"] = np.asarray(bv, dtype=np.float32).reshape(H, DH).mean(axis=0).astype(BF)
    bdv = bcontrasts(bv) * np.float32(WS)
    bb["d"] = np.concatenate(
        [bdv[0:NO], bdv[DH : DH + NO], bdv[NO:DH], bdv[DH + NO :]]
    ).astype(BF)
    for nm in bb:
        bb[nm] = np.ascontiguousarray(np.broadcast_to(bb[nm], (P, bb[nm].shape[0])))
    return ws, bb


def _prep_act8(a, bs):
    """fp8 DoubleRow: a8[bt, p, k8, i, b] = a[bt*128+b, (k8*2+i)*128+p]."""
    nbt = bs // P
    a8 = a.astype(E4).reshape(nbt, P, K8, 2, P).transpose(0, 4, 2, 3, 1)
    return np.ascontiguousarray(a8)


def _prep_act16(a, bs):
    """bf16: a16[bt, p, k, b] = a[bt*128+b, k*128+p]."""
    nbt = bs // P
    a16 = a.astype(BF).reshape(nbt, P, KT, P).transpose(0, 3, 2, 1)
    return np.ascontiguousarray(a16)


def _in_maps(x, sa, ws, bb, bs):
    maps = []
    for c in range(NCORES):
        r0 = c * bs
        maps.append(
            {
                "sa8": _prep_act8(sa[r0 : r0 + bs], bs),
                "x8": _prep_act8(x[r0 : r0 + bs], bs),
                "x16": _prep_act16(x[r0 : r0 + bs], bs),
                "wq8": ws["q"],
                "we8": ws["e"],
                "wm16": ws["m"],
                "wd8": ws["d"],
                "bq128": bb["q"],
                "be128": bb["e"],
                "bm": bb["m"],
                "bd128": bb["d"],
            }
        )
    return maps


def kernel(x, synthetic_attributes, Wq, bq, Wk, bk, Wv, bv, **_ignored):
    from concourse import bass_utils

    x = np.asarray(x, dtype=np.float32)
    sa = np.asarray(synthetic_attributes, dtype=np.float32)
    bs = x.shape[0] // NCORES

    ws, bb = _prep_weights(Wq, Wk, Wv, bq, bk, bv)
    nc = _get_nc(bs=bs)
    in_maps = _in_maps(x, sa, ws, bb, bs)

    res = bass_utils.run_bass_kernel_spmd(nc, in_maps, core_ids=list(range(NCORES)))
    out = np.concatenate([res.results[c]["out"] for c in range(NCORES)], axis=0)
    return out
